# revision 29
# baseline (speedup 1.0000x reference)
"""Trainium2 Bass kernel: out = x @ ((W_int + offset) * scale).

Math: out[m,n] = scale[n] * ((x @ (W-63))[m,n] + (63+offset[n]) * rowsum(x)[m]),
so the dequantized weight is never materialized. The centered weight
W-63 (ints in [-63,63]) and x are quantized to fp8 e4m3 and the matmul
runs in DoubleRow mode: each PE cell holds 2 fp8 weights, one matmul
contracts 256 K-elements — 2x the bf16 MAC rate (measured DR matmul dur
== bf16 dur at equal output width). KB trailing K-blocks (of 32) can run
in bf16 to trim quantization error; at NF=16 (pure fp8) the measured rel
err is 1.845e-2 vs the float64 reference (gate 2e-2), numpy-model-exact
and deterministic. PE-stream floor: 512 (m,ko256) pairs x 1376 out-cols
/ 2.4GHz = 293.5us; measured ~324us HW exec on 8 cores (~1.94x over the
628us bf16 baseline). Runs occasionally power-throttle to 2.0GHz (~20%
slower) — that is chip state, not kernel behavior.

Sharding: column-parallel — W / scale / offset / out split along N across
8 cores; x (as fp8 x^T pair layout) replicated.

Per-core kernel: the whole W shard is cached in SBUF as NF fp8
pair-blocks [128, 2, 1376] (352KB units; the first two split per-n-tile
so the first matmul starts ~2us sooner) + KB bf16 blocks; x m-tiles
(host-retiled so each load is 128 x 4KB contiguous) stream with
double-buffering; PSUM accumulates over NF+KB delivery units per output
tile. During the HBM-bound W-load phase, three m-tile groups consume
the units with staggered lags (8 PSUM banks) so the PE tracks the
delivery rate instead of idling; remaining m-tiles then stream at the
DoubleRow roofline.
"""

import numpy as np
import ml_dtypes

M, K, N = 4096, 4096, 11008
NCORES = 8
NSH = N // NCORES  # 1376
P = 128
KO = K // P        # 32
MO = M // P        # 32
N_TILES = [(0, 512), (512, 512), (1024, 352)]

NF = 16            # fp8 DoubleRow pair-blocks (cover 2*NF K-blocks)
KB = KO - 2 * NF   # trailing bf16 K-blocks
NU = NF + KB       # W delivery units (352KB each)

_BF16 = ml_dtypes.bfloat16
_E4 = ml_dtypes.float8_e4m3

_cache = {}


def _build_nc():
    import concourse.bacc as bacc
    import concourse.mybir as mybir
    import concourse.tile as tile

    bf16 = mybir.dt.bfloat16
    f32 = mybir.dt.float32
    fp8 = mybir.dt.float8e4
    DR = mybir.MatmulPerfMode.DoubleRow

    nc = bacc.Bacc(None, target_bir_lowering=False)
    # xt8 is host-retiled fp8 pairs: xt8[mo*P + ki, j*256 + s*128 + c]
    #   = e4m3(x)[mo*P + c, (2j+s)*128 + ki]
    xt8 = nc.dram_tensor("xt8", [M, NF * 2 * P], fp8, kind="ExternalInput")
    if KB:
        # xtb[mo*P + ki, kob*128 + c] = bf16(x)[mo*P + c, (2NF+kob)*128 + ki]
        xtb = nc.dram_tensor("xtb", [M, KB * P], bf16, kind="ExternalInput")
    # w8[j*P + ki, s*NSH + n] = e4m3(W - 63)[(2j+s)*128 + ki, n_shard]
    w8 = nc.dram_tensor("w8", [NF * P, 2 * NSH], fp8, kind="ExternalInput")
    if KB:
        wb = nc.dram_tensor("wb", [KB * P, NSH], bf16, kind="ExternalInput")
    scaleb = nc.dram_tensor("scaleb", [P, NSH], f32, kind="ExternalInput")
    offb = nc.dram_tensor("offb", [P, NSH], f32, kind="ExternalInput")
    scol = nc.dram_tensor("scol", [P, MO], f32, kind="ExternalInput")
    out = nc.dram_tensor("out", [M, NSH], f32, kind="ExternalOutput")

    xt8_3 = xt8.ap().rearrange("(mo p) f -> p mo f", p=P)   # [128, 32, NF*256]
    w8_3 = w8.ap().rearrange("(j p) f -> p j f", p=P)       # [128, NF, 2*NSH]
    if KB:
        xtb_3 = xtb.ap().rearrange("(mo p) f -> p mo f", p=P)
        wb_3 = wb.ap().rearrange("(ko p) n -> p ko n", p=P)
    out3 = out.ap().rearrange("(mo p) n -> p mo n", p=P)    # [128, 32, 1376]

    with tile.TileContext(nc) as tc:
        with (
            tc.tile_pool(name="wpool", bufs=1) as wpool,
            tc.tile_pool(name="xpool", bufs=5) as xpool,
            tc.tile_pool(name="opool", bufs=3) as opool,
            tc.tile_pool(name="cpool", bufs=1) as cpool,
            tc.tile_pool(name="psp", bufs=8, space="PSUM") as psp,
        ):
            x_tiles = {}

            def load_x(mo, split=False):
                t = xpool.tile([P, NF, 2, P], fp8, tag="x8")
                src = xt8_3[:, mo, :].rearrange("p (j s c) -> p j s c", s=2, c=P)
                if split:
                    # halves on two queues: the first m-tile lands ~2x sooner
                    h = NF // 2
                    nc.sync.dma_start(t[:, :h], src[:, :h])
                    nc.gpsimd.dma_start(t[:, h:], src[:, h:])
                else:
                    nc.sync.dma_start(t[:], src)
                if KB:
                    tb = xpool.tile([P, KB, P], bf16, tag="xb")
                    nc.sync.dma_start(
                        tb[:],
                        xtb_3[:, mo, :].rearrange("p (ko c) -> p ko c", c=P),
                    )
                else:
                    tb = None
                x_tiles[mo] = (t, tb)

            # First three x m-tiles up front (they run unit-synchronous with
            # the W-unit arrival during the W-load phase).
            load_x(0, split=True)
            load_x(1)
            load_x(2)

            # W shard: NU contiguous 352KB units spread across the DMA
            # issuers. Early units avoid the sync queue (busy with x0-x2)
            # so they arrive early; sync joins later. The first two units
            # are split per-n-tile (DMA-region tracking lets their first
            # matmuls start before the whole unit lands). Epilogue
            # constants go to queue tails.
            w8_sb = []
            for j in range(NF):
                t = wpool.tile([P, 2, NSH], fp8, tag=f"w8_{j}")
                if j < 6:
                    eng = [nc.scalar, nc.gpsimd][j % 2]
                else:
                    eng = [nc.scalar, nc.gpsimd, nc.sync][j % 3]
                src = w8_3[:, j, :].rearrange("p (s n) -> p s n", s=2)
                if j < 2:
                    for n0, nw in N_TILES:
                        eng.dma_start(t[:, :, n0:n0 + nw], src[:, :, n0:n0 + nw])
                else:
                    eng.dma_start(t[:], src)
                w8_sb.append(t)
            wb_sb = []
            for ko in range(KB):
                t = wpool.tile([P, NSH], bf16, tag=f"wb_{ko}")
                eng = [nc.scalar, nc.gpsimd, nc.sync][(NF + ko) % 3]
                eng.dma_start(t[:], wb_3[:, ko, :])
                wb_sb.append(t)

            scale_sb = cpool.tile([P, NSH], f32, tag="scale")
            nc.sync.dma_start(scale_sb[:], scaleb.ap())
            off_sb = cpool.tile([P, NSH], f32, tag="off")
            nc.gpsimd.dma_start(off_sb[:], offb.ap())
            scol_sb = cpool.tile([P, MO], f32, tag="scol")
            nc.gpsimd.dma_start(scol_sb[:], scol.ap())

            load_x(3)
            load_x(4)

            def unit_mm(g, u, ntiles, pss):
                """One W delivery unit's matmuls for m-tile g into pss."""
                x8_sb, xb_sb = x_tiles[g]
                for ti, (n0, nw) in enumerate(ntiles):
                    if u < NF:
                        nc.tensor.matmul(
                            pss[ti][:, :nw],
                            x8_sb[:, u, :, :],
                            w8_sb[u][:, :, n0:n0 + nw],
                            start=(u == 0),
                            stop=(u == NU - 1),
                            perf_mode=DR,
                        )
                    else:
                        ko = u - NF
                        nc.tensor.matmul(
                            pss[ti][:, :nw],
                            xb_sb[:, ko, :],
                            wb_sb[ko][:, n0:n0 + nw],
                            start=(u == 0),
                            stop=(u == NU - 1),
                        )

            def epilogue(mo, ps_tiles):
                o_sb = opool.tile([P, NSH], f32, tag="o")
                for ti, (n0, nw) in enumerate(N_TILES):
                    ps = ps_tiles[ti]
                    # ps += (63+offset[n]) * s[m]   (rank-1 term, fused DVE op)
                    nc.vector.scalar_tensor_tensor(
                        ps[:, :nw],
                        off_sb[:, n0:n0 + nw],
                        scol_sb[:, mo:mo + 1],
                        ps[:, :nw],
                        mybir.AluOpType.mult,
                        mybir.AluOpType.add,
                    )
                    # out = ps * scale[n]
                    nc.vector.tensor_mul(
                        out=o_sb[:, n0:n0 + nw],
                        in0=ps[:, :nw],
                        in1=scale_sb[:, n0:n0 + nw],
                    )
                    # store per n-tile so the tail overlaps the epilogue
                    nc.scalar.dma_start(
                        out3[:, mo, n0:n0 + nw], o_sb[:, n0:n0 + nw]
                    )

            # Phase 1: W delivery is HBM-bandwidth-bound, so three m-tile
            # groups consume the unit stream in issue order (PSUM
            # accumulation is order-agnostic) with staggered lags: m-tile 0
            # tracks the frontier, m-tile 1 runs LAG1 units behind, m-tile 2
            # (its two 512 n-tiles; 2 spare PSUM banks) LAG2 behind. The lag
            # keeps already-arrived units available whenever a fresh unit is
            # late, so the PE stays busy through the whole load phase.
            U_SORTED = list(range(NU))
            LAG1, LAG2 = 3, 7
            ps_f = [
                [
                    psp.tile([P, 512], mybir.dt.float32, tag="ps",
                             name=f"ps_f{g}_{ti}")
                    for ti in range(len(N_TILES))
                ]
                for g in range(2)
            ]
            ps_j = [
                psp.tile([P, 512], mybir.dt.float32, tag="ps", name=f"ps_j{ti}")
                for ti in range(2)
            ]

            for si in range(NU + LAG2):
                for g, lag in ((0, 0), (1, LAG1), (2, LAG2)):
                    idx = si - lag
                    if 0 <= idx < NU:
                        u = U_SORTED[idx]
                        ntiles = N_TILES if g < 2 else N_TILES[:2]
                        pss = ps_f[g] if g < 2 else ps_j
                        unit_mm(g, u, ntiles, pss)
            for g in range(2):
                epilogue(g, ps_f[g])
                x_tiles.pop(g)

            # m-tile 2's third n-tile (all W cached by now).
            n0_2, nw_2 = N_TILES[2]
            ps_2 = psp.tile([P, 512], mybir.dt.float32, tag="ps", name="ps_m2t2")
            for u in range(NU):
                unit_mm(2, u, [(n0_2, nw_2)], {0: ps_2})
            epilogue(2, [ps_j[0], ps_j[1], ps_2])
            x_tiles.pop(2)

            # Phase 2: remaining m-tiles, streaming.
            for mo in range(3, MO):
                if mo + 2 < MO:
                    load_x(mo + 2)
                ps_tiles = []
                for n0, nw in N_TILES:
                    ps = psp.tile([P, 512], mybir.dt.float32, tag="ps")
                    for u in range(NU):
                        unit_mm(mo, u, [(n0, nw)], {0: ps})
                    ps_tiles.append(ps)
                epilogue(mo, ps_tiles)
                x_tiles.pop(mo)
    nc.compile()
    return nc


def _get_nc():
    if "nc" not in _cache:
        _cache["nc"] = _build_nc()
    return _cache["nc"]


# e4m3 byte lookup for centered weights: _W_LUT[w] = e4m3(w - 63) bits
_W_LUT = (np.arange(127, dtype=np.float32) - 63.0).astype(_E4).view(np.uint8)


def _prep_inputs(x, weight, antiquant_scale, antiquant_offset):
    x = np.asarray(x, dtype=np.float32)
    weight = np.asarray(weight)
    antiquant_scale = np.asarray(antiquant_scale, dtype=np.float32)
    antiquant_offset = np.asarray(antiquant_offset, dtype=np.float32)

    x8 = x.astype(_E4)
    # xt8[mo, ki, j, s, c] = x8[mo*P + c, (2j+s)*128 + ki]
    xt8 = np.ascontiguousarray(
        x8.reshape(MO, P, NF, 2, P).transpose(0, 4, 2, 3, 1)
    ).reshape(M, NF * 2 * P)
    if KB:
        xb = x[:, 2 * NF * P:].astype(_BF16)
        xtb = np.ascontiguousarray(
            xb.reshape(MO, P, KB, P).transpose(0, 3, 2, 1)
        ).reshape(M, KB * P)
    s = x.sum(axis=1, dtype=np.float32)                      # [M]
    scol = np.ascontiguousarray(s.reshape(MO, P).T)          # [P, MO]

    w8_bits = _W_LUT[weight]                                 # [K, N] uint8
    in_maps = []
    for c in range(NCORES):
        sl = slice(c * NSH, (c + 1) * NSH)
        # w8c[j, ki, s, n] = e4m3(W-63)[(2j+s)*128 + ki, n]
        w8c = np.ascontiguousarray(
            w8_bits[:2 * NF * P, sl].reshape(NF, 2, P, NSH).transpose(0, 2, 1, 3)
        ).reshape(NF * P, 2 * NSH).view(_E4)
        im = {"xt8": xt8, "w8": w8c, "scol": scol}
        if KB:
            wbc = np.ascontiguousarray(
                (weight[2 * NF * P:, sl].astype(np.float32) - 63.0).astype(_BF16)
            )
            im["xtb"] = xtb
            im["wb"] = wbc
        im["scaleb"] = np.ascontiguousarray(
            np.broadcast_to(antiquant_scale[sl][None, :], (P, NSH))
        )
        im["offb"] = np.ascontiguousarray(
            np.broadcast_to(antiquant_offset[sl][None, :] + 63.0, (P, NSH))
        )
        in_maps.append(im)
    return in_maps


def kernel(x, weight, antiquant_scale, antiquant_offset, _trace=False):
    from concourse.bass_utils import run_bass_kernel_spmd

    nc = _get_nc()
    in_maps = _prep_inputs(x, weight, antiquant_scale, antiquant_offset)
    res = run_bass_kernel_spmd(
        nc, in_maps, core_ids=list(range(NCORES)), trace=_trace
    )
    out = np.concatenate([res.results[c]["out"] for c in range(NCORES)], axis=1)
    if _trace:
        _cache["last_result"] = res
    return out


# revision 31
# speedup vs baseline: 1.0183x; 1.0183x over previous
"""Trainium2 Bass kernel: out = x @ ((W_int + offset) * scale).

Math: out[m,n] = scale[n] * ((x @ (W-63))[m,n] + (63+offset[n]) * rowsum(x)[m]),
so the dequantized weight is never materialized. The centered weight
W-63 (ints in [-63,63]) and x are quantized to fp8 e4m3 and the matmul
runs in DoubleRow mode: each PE cell holds 2 fp8 weights, one matmul
contracts 256 K-elements — 2x the bf16 MAC rate (measured DR matmul dur
== bf16 dur at equal output width). KB trailing K-blocks (of 32) can run
in bf16 to trim quantization error; at NF=16 (pure fp8) the measured rel
err is 1.845e-2 vs the float64 reference (gate 2e-2), numpy-model-exact
and deterministic. PE-stream floor: 512 (m,ko256) pairs x 1376 out-cols
/ 2.4GHz = 293.5us; measured ~324us HW exec on 8 cores (~1.94x over the
628us bf16 baseline). Runs occasionally power-throttle to 2.0GHz (~20%
slower) — that is chip state, not kernel behavior.

Sharding: column-parallel — W / scale / offset / out split along N across
8 cores; x (as fp8 x^T pair layout) replicated.

Per-core kernel: the whole W shard is cached in SBUF as NF fp8
pair-blocks [128, 2, 1376] (352KB units; the first two split per-n-tile
so the first matmul starts ~2us sooner) + KB bf16 blocks; x m-tiles
(host-retiled so each load is 128 x 4KB contiguous) stream with
double-buffering; PSUM accumulates over NF+KB delivery units per output
tile. During the HBM-bound W-load phase, three m-tile groups consume
the units with staggered lags (8 PSUM banks) so the PE tracks the
delivery rate instead of idling; remaining m-tiles then stream at the
DoubleRow roofline.
"""

import numpy as np
import ml_dtypes

M, K, N = 4096, 4096, 11008
NCORES = 8
NSH = N // NCORES  # 1376
P = 128
KO = K // P        # 32
MO = M // P        # 32
N_TILES = [(0, 512), (512, 512), (1024, 352)]

NF = 16            # fp8 DoubleRow pair-blocks (cover 2*NF K-blocks)
KB = KO - 2 * NF   # trailing bf16 K-blocks
NU = NF + KB       # W delivery units (352KB each)

_BF16 = ml_dtypes.bfloat16
_E4 = ml_dtypes.float8_e4m3

_cache = {}


def _build_nc():
    import concourse.bacc as bacc
    import concourse.mybir as mybir
    import concourse.tile as tile

    bf16 = mybir.dt.bfloat16
    f32 = mybir.dt.float32
    fp8 = mybir.dt.float8e4
    DR = mybir.MatmulPerfMode.DoubleRow

    nc = bacc.Bacc(None, target_bir_lowering=False)
    # xt8 is host-retiled fp8 pairs: xt8[mo*P + ki, j*256 + s*128 + c]
    #   = e4m3(x)[mo*P + c, (2j+s)*128 + ki]
    xt8 = nc.dram_tensor("xt8", [M, NF * 2 * P], fp8, kind="ExternalInput")
    if KB:
        # xtb[mo*P + ki, kob*128 + c] = bf16(x)[mo*P + c, (2NF+kob)*128 + ki]
        xtb = nc.dram_tensor("xtb", [M, KB * P], bf16, kind="ExternalInput")
    # w8[j*P + ki, s*NSH + n] = e4m3(W - 63)[(2j+s)*128 + ki, n_shard]
    w8 = nc.dram_tensor("w8", [NF * P, 2 * NSH], fp8, kind="ExternalInput")
    if KB:
        wb = nc.dram_tensor("wb", [KB * P, NSH], bf16, kind="ExternalInput")
    scaleb = nc.dram_tensor("scaleb", [P, NSH], f32, kind="ExternalInput")
    offb = nc.dram_tensor("offb", [P, NSH], f32, kind="ExternalInput")
    scol = nc.dram_tensor("scol", [P, MO], f32, kind="ExternalInput")
    out = nc.dram_tensor("out", [M, NSH], f32, kind="ExternalOutput")

    xt8_3 = xt8.ap().rearrange("(mo p) f -> p mo f", p=P)   # [128, 32, NF*256]
    w8_3 = w8.ap().rearrange("(j p) f -> p j f", p=P)       # [128, NF, 2*NSH]
    if KB:
        xtb_3 = xtb.ap().rearrange("(mo p) f -> p mo f", p=P)
        wb_3 = wb.ap().rearrange("(ko p) n -> p ko n", p=P)
    out3 = out.ap().rearrange("(mo p) n -> p mo n", p=P)    # [128, 32, 1376]

    with tile.TileContext(nc) as tc:
        with (
            tc.tile_pool(name="wpool", bufs=1) as wpool,
            tc.tile_pool(name="xpool", bufs=5) as xpool,
            tc.tile_pool(name="opool", bufs=3) as opool,
            tc.tile_pool(name="cpool", bufs=1) as cpool,
            tc.tile_pool(name="psp", bufs=8, space="PSUM") as psp,
        ):
            x_tiles = {}

            def load_x(mo, split=False):
                t = xpool.tile([P, NF, 2, P], fp8, tag="x8")
                src = xt8_3[:, mo, :].rearrange("p (j s c) -> p j s c", s=2, c=P)
                if split:
                    # halves on two queues: the first m-tile lands ~2x sooner
                    h = NF // 2
                    nc.sync.dma_start(t[:, :h], src[:, :h])
                    nc.gpsimd.dma_start(t[:, h:], src[:, h:])
                else:
                    nc.sync.dma_start(t[:], src)
                if KB:
                    tb = xpool.tile([P, KB, P], bf16, tag="xb")
                    nc.sync.dma_start(
                        tb[:],
                        xtb_3[:, mo, :].rearrange("p (ko c) -> p ko c", c=P),
                    )
                else:
                    tb = None
                x_tiles[mo] = (t, tb)

            # First three x m-tiles up front (they run unit-synchronous with
            # the W-unit arrival during the W-load phase).
            load_x(0, split=True)
            load_x(1)
            load_x(2)

            # W shard: NU contiguous 352KB units spread across the DMA
            # issuers. Early units avoid the sync queue (busy with x0-x2)
            # so they arrive early; sync joins later. Epilogue constants
            # go to queue tails. (Splitting units into per-n-tile chunk
            # DMAs was tried and is a net loss: each dma_start costs its
            # sequencer ~0.7us of issue time, delaying successor units.)
            w8_sb = []
            for j in range(NF):
                t = wpool.tile([P, 2, NSH], fp8, tag=f"w8_{j}")
                if j < 6:
                    eng = [nc.scalar, nc.gpsimd][j % 2]
                else:
                    eng = [nc.scalar, nc.gpsimd, nc.sync][j % 3]
                eng.dma_start(t[:], w8_3[:, j, :].rearrange("p (s n) -> p s n", s=2))
                w8_sb.append(t)
            wb_sb = []
            for ko in range(KB):
                t = wpool.tile([P, NSH], bf16, tag=f"wb_{ko}")
                eng = [nc.scalar, nc.gpsimd, nc.sync][(NF + ko) % 3]
                eng.dma_start(t[:], wb_3[:, ko, :])
                wb_sb.append(t)

            scale_sb = cpool.tile([P, NSH], f32, tag="scale")
            nc.sync.dma_start(scale_sb[:], scaleb.ap())
            off_sb = cpool.tile([P, NSH], f32, tag="off")
            nc.gpsimd.dma_start(off_sb[:], offb.ap())
            scol_sb = cpool.tile([P, MO], f32, tag="scol")
            nc.gpsimd.dma_start(scol_sb[:], scol.ap())

            load_x(3)
            load_x(4)

            def unit_mm(g, u, ntiles, pss):
                """One W delivery unit's matmuls for m-tile g into pss."""
                x8_sb, xb_sb = x_tiles[g]
                for ti, (n0, nw) in enumerate(ntiles):
                    if u < NF:
                        nc.tensor.matmul(
                            pss[ti][:, :nw],
                            x8_sb[:, u, :, :],
                            w8_sb[u][:, :, n0:n0 + nw],
                            start=(u == 0),
                            stop=(u == NU - 1),
                            perf_mode=DR,
                        )
                    else:
                        ko = u - NF
                        nc.tensor.matmul(
                            pss[ti][:, :nw],
                            xb_sb[:, ko, :],
                            wb_sb[ko][:, n0:n0 + nw],
                            start=(u == 0),
                            stop=(u == NU - 1),
                        )

            def epilogue(mo, ps_tiles):
                o_sb = opool.tile([P, NSH], f32, tag="o")
                for ti, (n0, nw) in enumerate(N_TILES):
                    ps = ps_tiles[ti]
                    # ps += (63+offset[n]) * s[m]   (rank-1 term, fused DVE op)
                    nc.vector.scalar_tensor_tensor(
                        ps[:, :nw],
                        off_sb[:, n0:n0 + nw],
                        scol_sb[:, mo:mo + 1],
                        ps[:, :nw],
                        mybir.AluOpType.mult,
                        mybir.AluOpType.add,
                    )
                    # out = ps * scale[n]
                    nc.vector.tensor_mul(
                        out=o_sb[:, n0:n0 + nw],
                        in0=ps[:, :nw],
                        in1=scale_sb[:, n0:n0 + nw],
                    )
                    # store per n-tile so the tail overlaps the epilogue
                    nc.scalar.dma_start(
                        out3[:, mo, n0:n0 + nw], o_sb[:, n0:n0 + nw]
                    )

            # Phase 1: W delivery is HBM-bandwidth-bound, so three m-tile
            # groups consume the unit stream in issue order (PSUM
            # accumulation is order-agnostic) with staggered lags: m-tile 0
            # tracks the frontier, m-tile 1 runs LAG1 units behind, m-tile 2
            # (its two 512 n-tiles; 2 spare PSUM banks) LAG2 behind. The lag
            # keeps already-arrived units available whenever a fresh unit is
            # late, so the PE stays busy through the whole load phase.
            U_SORTED = list(range(NU))
            LAG1, LAG2 = 3, 7
            ps_f = [
                [
                    psp.tile([P, 512], mybir.dt.float32, tag="ps",
                             name=f"ps_f{g}_{ti}")
                    for ti in range(len(N_TILES))
                ]
                for g in range(2)
            ]
            ps_j = [
                psp.tile([P, 512], mybir.dt.float32, tag="ps", name=f"ps_j{ti}")
                for ti in range(2)
            ]

            for si in range(NU + LAG2):
                for g, lag in ((0, 0), (1, LAG1), (2, LAG2)):
                    idx = si - lag
                    if 0 <= idx < NU:
                        u = U_SORTED[idx]
                        ntiles = N_TILES if g < 2 else N_TILES[:2]
                        pss = ps_f[g] if g < 2 else ps_j
                        unit_mm(g, u, ntiles, pss)
            for g in range(2):
                epilogue(g, ps_f[g])
                x_tiles.pop(g)

            # m-tile 2's third n-tile (all W cached by now).
            n0_2, nw_2 = N_TILES[2]
            ps_2 = psp.tile([P, 512], mybir.dt.float32, tag="ps", name="ps_m2t2")
            for u in range(NU):
                unit_mm(2, u, [(n0_2, nw_2)], {0: ps_2})
            epilogue(2, [ps_j[0], ps_j[1], ps_2])
            x_tiles.pop(2)

            # Phase 2: remaining m-tiles, streaming.
            for mo in range(3, MO):
                if mo + 2 < MO:
                    load_x(mo + 2)
                ps_tiles = []
                for n0, nw in N_TILES:
                    ps = psp.tile([P, 512], mybir.dt.float32, tag="ps")
                    for u in range(NU):
                        unit_mm(mo, u, [(n0, nw)], {0: ps})
                    ps_tiles.append(ps)
                epilogue(mo, ps_tiles)
                x_tiles.pop(mo)
    nc.compile()
    return nc


def _get_nc():
    if "nc" not in _cache:
        _cache["nc"] = _build_nc()
    return _cache["nc"]


# e4m3 byte lookup for centered weights: _W_LUT[w] = e4m3(w - 63) bits
_W_LUT = (np.arange(127, dtype=np.float32) - 63.0).astype(_E4).view(np.uint8)


def _prep_inputs(x, weight, antiquant_scale, antiquant_offset):
    x = np.asarray(x, dtype=np.float32)
    weight = np.asarray(weight)
    antiquant_scale = np.asarray(antiquant_scale, dtype=np.float32)
    antiquant_offset = np.asarray(antiquant_offset, dtype=np.float32)

    x8 = x.astype(_E4)
    # xt8[mo, ki, j, s, c] = x8[mo*P + c, (2j+s)*128 + ki]
    xt8 = np.ascontiguousarray(
        x8.reshape(MO, P, NF, 2, P).transpose(0, 4, 2, 3, 1)
    ).reshape(M, NF * 2 * P)
    if KB:
        xb = x[:, 2 * NF * P:].astype(_BF16)
        xtb = np.ascontiguousarray(
            xb.reshape(MO, P, KB, P).transpose(0, 3, 2, 1)
        ).reshape(M, KB * P)
    s = x.sum(axis=1, dtype=np.float32)                      # [M]
    scol = np.ascontiguousarray(s.reshape(MO, P).T)          # [P, MO]

    w8_bits = _W_LUT[weight]                                 # [K, N] uint8
    in_maps = []
    for c in range(NCORES):
        sl = slice(c * NSH, (c + 1) * NSH)
        # w8c[j, ki, s, n] = e4m3(W-63)[(2j+s)*128 + ki, n]
        w8c = np.ascontiguousarray(
            w8_bits[:2 * NF * P, sl].reshape(NF, 2, P, NSH).transpose(0, 2, 1, 3)
        ).reshape(NF * P, 2 * NSH).view(_E4)
        im = {"xt8": xt8, "w8": w8c, "scol": scol}
        if KB:
            wbc = np.ascontiguousarray(
                (weight[2 * NF * P:, sl].astype(np.float32) - 63.0).astype(_BF16)
            )
            im["xtb"] = xtb
            im["wb"] = wbc
        im["scaleb"] = np.ascontiguousarray(
            np.broadcast_to(antiquant_scale[sl][None, :], (P, NSH))
        )
        im["offb"] = np.ascontiguousarray(
            np.broadcast_to(antiquant_offset[sl][None, :] + 63.0, (P, NSH))
        )
        in_maps.append(im)
    return in_maps


def kernel(x, weight, antiquant_scale, antiquant_offset, _trace=False):
    from concourse.bass_utils import run_bass_kernel_spmd

    nc = _get_nc()
    in_maps = _prep_inputs(x, weight, antiquant_scale, antiquant_offset)
    res = run_bass_kernel_spmd(
        nc, in_maps, core_ids=list(range(NCORES)), trace=_trace
    )
    out = np.concatenate([res.results[c]["out"] for c in range(NCORES)], axis=1)
    if _trace:
        _cache["last_result"] = res
    return out
